# revision 65
# baseline (speedup 1.0000x reference)
"""Distributed sparse-attention kernel for 8 TRN2 NeuronCores.

Sharding: Megatron-style head parallelism. Core c owns heads [4c, 4c+4):
Wq/Wk/Wv column-parallel (rows of the [H*DH, D] weights), Wo row-parallel
(columns of [D, H*DH]). Each core computes a partial output
out_c = Wo_c @ ctx_c over its heads; the host sums the 8 partials.

Device layout choices (no on-chip transposes anywhere):
  h_q, h_k   : [dh, len] fp16  (proj psum M=dh-chunk, N=len)
  h_vT       : [k, dh+1] bf16  (proj psum M=k-chunk, N=dh; ones column
               appended so the ctx matmul also emits the softmax denom)
  scores     : [k, q] psum; pb+mask bias accumulated into the same psum
               bank by an identity-matmul (out += I.T @ pbm), exp on ACT
               straight from psum. No max-subtraction: scores are O(50)
               so exp fits fp32/bf16 range, masked entries are -1e30 ->
               exp underflows to exact 0.
  ctx        : psum [65, q] f32; row 64 = sum_k exp  (denominator)
  normalize  : 1/denom as exp(-ln(x)) on ACT (batched per head-pair to
               avoid ACT function-table reloads), then gpsimd
               partition_broadcast + DVE mul -> ctxn bf16

Precision: Q/K path (projections + scores) in fp16 — scores feed exp(),
so absolute score error must stay ~1e-2; fp16 keeps it ~5e-3 while bf16
would give ~5e-2. V/ctx/output path errors only enter linearly, so bf16
is fine there (exp values overflow fp16 range, hence bf16 anyway).
V-projection packs two k-chunk accumulations per PSUM bank: the first
matmul's start=True clears has_written for the whole bank, the partner
k-chunk then starts with start=False and overwrites-where-unset.
The pb+mask add alternates between the PE (identity matmul) and the DVE
(tensor_add) by k-chunk parity to balance the two engines.
(Known-broken on HW, avoid: reciprocal_approx_fast and
partition_broadcast from a non-zero base partition — both return
garbage on silicon while passing CoreSim.)
"""

import sys

for _p in ("/opt/trn_rl_repo",):
    if _p not in sys.path:
        sys.path.insert(0, _p)

from contextlib import ExitStack

import numpy as np
import ml_dtypes

import concourse.bass as bass
import concourse.mybir as mybir
import concourse.tile as tile
from concourse import bacc
from concourse.bass_utils import run_bass_kernel_spmd

B, D, H, DH, LQ, LK = 2, 2048, 32, 64, 1024, 1024
NCORES = 8
HC = H // NCORES          # heads per core = 4
MR = HC * DH              # per-core model rows = 256
NEG = -1e30

DC = D // 128             # 16 d-chunks
NKC = LK // 128           # 8 k-chunks
NQB = LQ // 512           # 2 q blocks
NMC = MR // 128           # 2 dh-chunks
NOC = D // 128            # 16 output-row chunks

F32 = mybir.dt.float32
F32R = mybir.dt.float32r
F16 = mybir.dt.float16
BF16 = mybir.dt.bfloat16

USE_IDENT_ADD = True
CTX_LAG = 8
SPLIT_OUTPROJ = False

CFG = dict(
    dt_hid=F16,    # HBM hidden_q / hidden_kv (Q/K-path matmul operand)
    dt_w=F16,      # HBM Wq/Wk/Wv
    dt_wo=BF16,    # HBM Wo
    dt_pbm=BF16,   # HBM combined position_bias + mask bias
    dt_out=BF16,   # HBM partial output
    dt_qk=F16,     # SBUF h_q / h_k
    dt_v=BF16,     # SBUF h_vT / exp / ctxn
)

_NP = {F32: np.float32, F32R: np.float32, F16: np.float16,
       BF16: ml_dtypes.bfloat16}


def build_nc():
    dt_hid, dt_w, dt_wo = CFG["dt_hid"], CFG["dt_w"], CFG["dt_wo"]
    dt_pbm, dt_out = CFG["dt_pbm"], CFG["dt_out"]
    dt_qk, dt_v = CFG["dt_qk"], CFG["dt_v"]

    nc = bacc.Bacc("TRN2", target_bir_lowering=False, debug=False,
                   num_devices=NCORES)
    # hidden/pbm/weights are host-interleaved so each [128, 2*N] tile DMA
    # reads one contiguous 2*N-element run per partition (bigger packets)
    hq_e = nc.declare_dram_parameter("hq", [B, DC // 2, 128, 2 * LQ],
                                     dt_hid, False)
    hkv_e = nc.declare_dram_parameter("hkv", [B, DC // 2, 128, 2 * LK],
                                      dt_hid, False)
    pbm_e = nc.declare_dram_parameter("pbm", [B, HC, NKC // 2, 128, 2 * LQ],
                                      dt_pbm, False)
    wqt_e = nc.declare_dram_parameter("wqt", [DC // 4, 128, 4 * MR], dt_w,
                                      False)
    wkt_e = nc.declare_dram_parameter("wkt", [DC // 4, 128, 4 * MR], dt_w,
                                      False)
    wvt_e = nc.declare_dram_parameter("wvt", [DC // 4, 128, 4 * MR], dt_w,
                                      False)
    wot_e = nc.declare_dram_parameter("wot", [MR, D], dt_wo, False)
    id_e = nc.declare_dram_parameter("ident", [128, 128], dt_pbm, False)
    out_e = nc.declare_dram_parameter("out", [B, D, LQ], dt_out, True)
    hq_a, hkv_a, pbm_a, out_a = hq_e.ap(), hkv_e.ap(), pbm_e.ap(), out_e.ap()

    mm = nc.tensor.matmul
    Exp = mybir.ActivationFunctionType.Exp
    Ln = mybir.ActivationFunctionType.Ln

    with tile.TileContext(nc) as tc, ExitStack() as ctx:
        wp = ctx.enter_context(tc.tile_pool(name="w", bufs=1))
        hidp = ctx.enter_context(tc.tile_pool(name="hid", bufs=4))
        sbp = ctx.enter_context(tc.tile_pool(name="sb", bufs=1))
        pbmp = ctx.enter_context(tc.tile_pool(name="pbm", bufs=5))
        tmpp = ctx.enter_context(tc.tile_pool(name="tmp", bufs=4))
        expp = ctx.enter_context(tc.tile_pool(name="expp", bufs=8))
        psp = ctx.enter_context(tc.tile_pool(name="ps", bufs=4, space="PSUM"))

        # --- persistent weights, 4 d-chunks per DMA ----------------------
        # wX_sb[dc] are AP views into host-interleaved [128, 4*MR] tiles
        def load_w4(nm, ap_):
            views = []
            for g in range(DC // 4):
                t = wp.tile([128, 4 * MR], dt_w, tag=f"{nm}{g}", name=f"{nm}{g}")
                nc.sync.dma_start(t[:, :], ap_[g])
                for i in range(4):
                    views.append(t[:, i * MR:(i + 1) * MR])
            return views

        wk_sb, wv_sb = [], []
        pre_hkv, pre_pbm = {}, {}

        def wload(nm, ap_, lst, g):
            t = wp.tile([128, 4 * MR], dt_w, tag=f"{nm}{g}", name=f"{nm}{g}")
            nc.sync.dma_start(t[:, :], ap_[g])
            for i in range(4):
                lst.append(t[:, i * MR:(i + 1) * MR])

        def fetch_hkv(b, dc2, name):
            t = hidp.tile([128, 2 * LK], dt_hid, tag="hkv", name=name)
            nc.sync.dma_start(t[:, :], hkv_a[b, dc2 // 2])
            return t

        # startup: a 64 KB dc=0 slice of Wk and the first hidden pair
        # lead the DMA queue so the opening K matmuls start ASAP
        wk0s = wp.tile([128, MR], dt_w, tag="wk0s", name="wk0s")
        nc.sync.dma_start(wk0s[:, :], wkt_e.ap()[0, :, 0:MR])
        h0t = hidp.tile([128, 2 * LK], dt_hid, tag="hkv", name="hkvA0_0")
        nc.sync.dma_start(h0t[:, 0:LK], hkv_a[0, 0, :, 0:LK])
        nc.sync.dma_start(h0t[:, LK:2 * LK], hkv_a[0, 0, :, LK:2 * LK])
        pre_hkv[0] = h0t
        wload("wk", wkt_e.ap(), wk_sb, 0)
        wv0s = wp.tile([128, MR], dt_w, tag="wv0s", name="wv0s")
        nc.sync.dma_start(wv0s[:, :], wvt_e.ap()[0, :, 0:MR])
        wload("wv", wvt_e.ap(), wv_sb, 0)
        pre_hkv[2] = fetch_hkv(0, 2, "hkvA0_2")
        for g in range(1, DC // 4):
            wload("wk", wkt_e.ap(), wk_sb, g)
            wload("wv", wvt_e.ap(), wv_sb, g)
        wk_sb[0] = wk0s[:, :]
        wv_sb[0] = wv0s[:, :]
        ident = wp.tile([128, 128], dt_pbm, tag="ident", name="ident")
        nc.sync.dma_start(ident[:, :], id_e.ap()[:, :])
        ones1 = wp.tile([128, 1], F32, tag="ones1", name="ones1")
        nc.gpsimd.memset(ones1[:, :], 1.0)

        wq_sb, wo_sb = [], []

        def load_wq():
            wq_sb.extend(load_w4("wq", wqt_e.ap()))

        def load_wo():
            for c in range(NMC):
                t = wp.tile([128, D], dt_wo, tag=f"wo{c}", name=f"wo{c}")
                nc.sync.dma_start(t[:, :], wot_e.ap()[c * 128:(c + 1) * 128, :])
                wo_sb.append(t)

        ctxn_by_b = {}
        phases = {}

        for b in range(B):
            # --- pass A: K proj (4 banks) + V proj 2 k-chunks/bank (4) ---
            pk = [psp.tile([128, 512], F32, tag="big", name=f"pk{b}_{i}")
                  for i in range(4)]
            pv = [psp.tile([128, 512], F32, tag="pv", name=f"pv{b}_{i}")
                  for i in range(4)]
            hvT = [sbp.tile([128, HC * (DH + 1)], dt_v, tag=f"hv{kc}",
                            name=f"hv{b}_{kc}") for kc in range(NKC)]
            hk_sb = [sbp.tile([128, LK], dt_qk, tag=f"hk{mc}",
                              name=f"hk{b}_{mc}") for mc in range(NMC)]
            hq_sb = [sbp.tile([128, LQ], dt_qk, tag=f"hqs{mc}",
                              name=f"hqs{b}_{mc}") for mc in range(NMC)]

            for dc2 in range(0, DC, 2):
                if b == 0 and dc2 in pre_hkv:
                    hkv_t = pre_hkv.pop(dc2)
                else:
                    hkv_t = fetch_hkv(b, dc2, f"hkvA{b}_{dc2}")
                for i in range(2):
                    dc = dc2 + i
                    hv = hkv_t[:, i * LK:(i + 1) * LK]
                    for mc in range(NMC):
                        for kb in range(2):
                            mm(pk[mc * 2 + kb][:, :],
                               wk_sb[dc][:, mc * 128:(mc + 1) * 128],
                               hv[:, kb * 512:(kb + 1) * 512],
                               start=dc == 0, stop=dc == DC - 1)
                    for kc in range(NKC):
                        # two k-chunks share a psum bank; the first matmul
                        # clears the bank, the last one closes the group
                        mm(pv[kc // 2][:, (kc % 2) * MR:(kc % 2) * MR + MR],
                           hv[:, kc * 128:(kc + 1) * 128],
                           wv_sb[dc][:, :],
                           start=(dc == 0 and kc % 2 == 0),
                           stop=(dc == DC - 1 and kc % 2 == 1))
            for mc in range(NMC):
                for kb in range(2):
                    nc.vector.tensor_copy(hk_sb[mc][:, kb * 512:(kb + 1) * 512],
                                          pk[mc * 2 + kb][:, :])
            for kc in range(NKC):
                src = pv[kc // 2][:, (kc % 2) * MR:(kc % 2) * MR + MR]
                for h in range(HC):
                    nc.vector.tensor_copy(
                        hvT[kc][:, h * (DH + 1):h * (DH + 1) + DH],
                        src[:, h * DH:(h + 1) * DH])
                    nc.vector.tensor_copy(
                        hvT[kc][:, h * (DH + 1) + DH:(h + 1) * (DH + 1)],
                        ones1[:, :])

            # --- pass B: Q proj ------------------------------------------
            if b == 0:
                load_wq()
            # prefetch the first head-pair's pbm tiles so attention
            # doesn't stall on DMA right after the Q projection
            for h0 in range(2):
                t = pbmp.tile([128, 2 * LQ], dt_pbm, tag="pbm",
                              name=f"pbm{b}_{h0}_0p")
                nc.sync.dma_start(t[:, :], pbm_a[b, h0, 0])
                pre_pbm[(b, h0, 0)] = t
            pq = [psp.tile([128, 512], F32, tag="big", name=f"pq{b}_{i}")
                  for i in range(4)]
            for dc2 in range(0, DC, 2):
                hq_t = hidp.tile([128, 2 * LQ], dt_hid, tag="hq",
                                 name=f"hqB{b}_{dc2}")
                nc.sync.dma_start(hq_t[:, :], hq_a[b, dc2 // 2])
                for i in range(2):
                    dc = dc2 + i
                    hv = hq_t[:, i * LQ:(i + 1) * LQ]
                    for mc in range(NMC):
                        for qb in range(2):
                            mm(pq[mc * 2 + qb][:, :],
                               wq_sb[dc][:, mc * 128:(mc + 1) * 128],
                               hv[:, qb * 512:(qb + 1) * 512],
                               start=dc == 0, stop=dc == DC - 1)
            for mc in range(NMC):
                for qb in range(2):
                    nc.vector.tensor_copy(hq_sb[mc][:, qb * 512:(qb + 1) * 512],
                                          pq[mc * 2 + qb][:, :])

            # --- attention, head pairs ------------------------------------
            ctxn = [sbp.tile([128, LQ], dt_v, tag=f"ctxn{b}_{c}",
                             name=f"ctxn{b}_{c}") for c in range(NMC)]
            oacc = {}
            cxr = {}
            for hp in range(HC // 2):
                hc = hp
                heads = (2 * hp, 2 * hp + 1)
                pctx = {}
                for h in heads:
                    for qb in range(NQB):
                        pctx[(h, qb)] = psp.tile([DH + 1, 512], F32, tag="pv",
                                                 name=f"pctx{b}_{h}_{qb}")
                # ctx matmuls are emitted LAG units behind the score/exp
                # stream, so at a pair boundary the PE has fresh scores to
                # chew on instead of stalling on the previous pair's
                # normalize freeing a pctx slot
                pending = []

                def flush_ctx(limit):
                    while len(pending) > limit:
                        h2, qb2, kc_2, ex2 = pending.pop(0)
                        mm(pctx[(h2, qb2)][:, :],
                           hvT[kc_2][:, h2 * (DH + 1):(h2 + 1) * (DH + 1)],
                           ex2[:, :],
                           start=kc_2 == 0, stop=kc_2 == NKC - 1)

                for kc2 in range(0, NKC, 2):
                    pbm_t = {}
                    for h in heads:
                        key = (b, h, kc2)
                        if key in pre_pbm:
                            pbm_t[h] = pre_pbm.pop(key)
                        else:
                            pbm_t[h] = pbmp.tile([128, 2 * LQ], dt_pbm,
                                                 tag="pbm",
                                                 name=f"pbm{b}_{h}_{kc2}")
                            nc.sync.dma_start(pbm_t[h][:, :],
                                              pbm_a[b, h, kc2 // 2])
                    for i in range(2):
                        kc = kc2 + i
                        use_ident = USE_IDENT_ADD and kc % 2 == 0
                        for qb in range(NQB):
                            ps_t, ex_t = {}, {}
                            for h in heads:
                                po = (h % 2) * 64
                                ps_t[h] = psp.tile([128, 512], F32, tag="big",
                                                   name=f"ps{b}_{h}_{kc}_{qb}")
                                mm(ps_t[h][:, :],
                                   hk_sb[hc][po:po + 64,
                                             kc * 128:(kc + 1) * 128],
                                   hq_sb[hc][po:po + 64,
                                             qb * 512:(qb + 1) * 512],
                                   start=True, stop=not use_ident)
                            for h in heads:
                                if use_ident:
                                    mm(ps_t[h][:, :], ident[:, :],
                                       pbm_t[h][:, i * LQ + qb * 512:
                                                i * LQ + (qb + 1) * 512],
                                       start=False, stop=True)
                            for h in heads:
                                ex_t[h] = expp.tile([128, 512], dt_v, tag="exp",
                                                    bufs=8,
                                                    name=f"ex{b}_{h}_{kc}_{qb}")
                                if use_ident:
                                    nc.scalar.activation(ex_t[h][:, :],
                                                         ps_t[h][:, :], Exp)
                                else:
                                    tmq = expp.tile([128, 512], F32, tag="tmq",
                                                    bufs=6,
                                                    name=f"tq{b}_{h}_{kc}_{qb}")
                                    nc.vector.tensor_add(
                                        tmq[:, :], ps_t[h][:, :],
                                        pbm_t[h][:, i * LQ + qb * 512:
                                                 i * LQ + (qb + 1) * 512])
                                    nc.scalar.activation(ex_t[h][:, :],
                                                         tmq[:, :], Exp)
                            for h in heads:
                                pending.append((h, qb, kc, ex_t[h]))
                            flush_ctx(CTX_LAG)
                flush_ctx(0)
                # evacuate raw ctx+denominator to SBUF (~0.7us/copy),
                # freeing the pctx psum slots for the next pair immediately;
                # all normalization is deferred to the end of the batch
                # where it hides under the next batch's projections (or the
                # previous batch's deferred out-projection)
                for h in heads:
                    for qb in range(NQB):
                        t = tmpp.tile([DH + 1, 512], F32, tag="cxr", bufs=8,
                                      name=f"cxr{b}_{h}_{qb}")
                        nc.vector.tensor_copy(t[:, :], pctx[(h, qb)][:, :])
                        cxr[(h, qb)] = t

            # batch-end normalize, one qb group at a time so the
            # qb-major out-projection can start on qb=0 columns while
            # qb=1 is still normalizing; 1/x as exp(-ln(x)) on ACT
            for qb in range(NQB):
                hs_ = list(range(HC))
                rl_t, rc_t = {}, {}
                for h in hs_:
                    rl_t[h] = tmpp.tile([1, 512], F32, tag="rl", bufs=8,
                                        name=f"rl{b}_{h}_{qb}")
                    nc.scalar.activation(rl_t[h][:, :],
                                         cxr[(h, qb)][DH:DH + 1, :], Ln)
                for h in hs_:
                    rc_t[h] = tmpp.tile([1, 512], F32, tag="rc", bufs=8,
                                        name=f"rc{b}_{h}_{qb}")
                    nc.scalar.activation(rc_t[h][:, :], rl_t[h][:, :], Exp,
                                         scale=-1.0)
                for h in hs_:
                    po = (h % 2) * 64
                    bc = tmpp.tile([64, 512], F32, tag="bc", bufs=6,
                                   name=f"bc{b}_{h}_{qb}")
                    nc.gpsimd.partition_broadcast(bc[:, :], rc_t[h][:, :])
                    nc.vector.tensor_mul(
                        ctxn[(h // 2)][po:po + 64, qb * 512:(qb + 1) * 512],
                        cxr[(h, qb)][0:DH, :], bc[:, :])

            ctxn_by_b[b] = ctxn
            if b == 0:
                load_wo()

        def do_outproj(b):
            ctxn = ctxn_by_b[b]
            for qb in range(NQB):
                for oc in range(NOC):
                    po_t = psp.tile([128, 512], F32, tag="big",
                                    name=f"po{b}_{oc}_{qb}")
                    for c in range(NMC):
                        mm(po_t[:, :],
                           wo_sb[c][:, oc * 128:(oc + 1) * 128],
                           ctxn[c][:, qb * 512:(qb + 1) * 512],
                           start=c == 0, stop=c == NMC - 1)
                    osb = tmpp.tile([128, 512], dt_out, tag="osb",
                                    name=f"osb{b}_{oc}_{qb}")
                    nc.vector.tensor_copy(osb[:, :], po_t[:, :])
                    nc.sync.dma_start(
                        out_a[b, oc * 128:(oc + 1) * 128,
                              qb * 512:(qb + 1) * 512],
                        osb[:, :])

        for b in range(B):
            do_outproj(b)

    nc.compile()
    return nc


_NC_CACHE = None


def _get_nc():
    global _NC_CACHE
    if _NC_CACHE is None:
        _NC_CACHE = build_nc()
    return _NC_CACHE


def make_in_maps(hidden_q, hidden_kv, mask, position_bias, Wq, Wk, Wv, Wo):
    np_hid = _NP[CFG["dt_hid"]]
    np_w = _NP[CFG["dt_w"]]
    np_wo = _NP[CFG["dt_wo"]]
    np_pbm = _NP[CFG["dt_pbm"]]
    hidden_q = np.asarray(hidden_q, np.float32)
    hidden_kv = np.asarray(hidden_kv, np.float32)
    mask = np.asarray(mask)
    position_bias = np.asarray(position_bias, np.float32)
    Wq, Wk, Wv, Wo = (np.asarray(w, np.float32) for w in (Wq, Wk, Wv, Wo))

    maskb = np.where(mask != 0, np.float32(0), np.float32(NEG))  # [B, LK, LQ]

    def ilv_hid(x):
        # [B, D, L] -> [B, D/256, 128, 2*L]: partition p of chunk-pair g
        # holds rows g*256+p and g*256+128+p contiguously
        b_, d_, l_ = x.shape
        return np.ascontiguousarray(
            x.reshape(b_, d_ // 256, 2, 128, l_).transpose(0, 1, 3, 2, 4)
        ).reshape(b_, d_ // 256, 128, 2 * l_)

    def ilv_w(wt):
        # [D, MR] -> [D/512, 128, 4*MR]
        d_, m_ = wt.shape
        return np.ascontiguousarray(
            wt.reshape(d_ // 512, 4, 128, m_).transpose(0, 2, 1, 3)
        ).reshape(d_ // 512, 128, 4 * m_)

    hq = ilv_hid(hidden_q.astype(np_hid))
    hkv = ilv_hid(hidden_kv.astype(np_hid))
    ident = np.eye(128, dtype=np_pbm)
    in_maps = []
    for c in range(NCORES):
        hs = slice(c * HC, (c + 1) * HC)
        rs = slice(c * MR, (c + 1) * MR)
        pbm = (position_bias[hs][None] + maskb[:, None]).astype(np_pbm)
        pbm = np.ascontiguousarray(
            pbm.reshape(B, HC, NKC // 2, 2, 128, LQ).transpose(0, 1, 2, 4, 3, 5)
        ).reshape(B, HC, NKC // 2, 128, 2 * LQ)
        in_maps.append({
            "hq": hq,
            "hkv": hkv,
            "pbm": pbm,
            "wqt": ilv_w(np.ascontiguousarray(Wq[rs].T).astype(np_w)),
            "wkt": ilv_w(np.ascontiguousarray(Wk[rs].T).astype(np_w)),
            "wvt": ilv_w(np.ascontiguousarray(Wv[rs].T).astype(np_w)),
            "wot": np.ascontiguousarray(Wo[:, rs].T).astype(np_wo),
            "ident": ident,
        })
    return in_maps


def run(in_maps, trace=False):
    nc = _get_nc()
    return run_bass_kernel_spmd(nc, in_maps, core_ids=list(range(NCORES)),
                                trace=trace)


def kernel(hidden_q, hidden_kv, mask, position_bias, Wq, Wk, Wv, Wo):
    in_maps = make_in_maps(hidden_q, hidden_kv, mask, position_bias,
                           Wq, Wk, Wv, Wo)
    res = run(in_maps, trace=False)
    acc = np.zeros((B, D, LQ), np.float32)
    for r in res.results:
        acc += np.asarray(r["out"], dtype=np.float32)
    return acc


# revision 66
# speedup vs baseline: 1.0618x; 1.0618x over previous
"""Distributed sparse-attention kernel for 8 TRN2 NeuronCores.

Sharding: Megatron-style head parallelism. Core c owns heads [4c, 4c+4):
Wq/Wk/Wv column-parallel (rows of the [H*DH, D] weights), Wo row-parallel
(columns of [D, H*DH]). Each core computes a partial output
out_c = Wo_c @ ctx_c over its heads; the host sums the 8 partials.

Device layout choices (no on-chip transposes anywhere):
  h_q, h_k   : [dh, len] fp16  (proj psum M=dh-chunk, N=len)
  h_vT       : [k, dh+1] bf16  (proj psum M=k-chunk, N=dh; ones column
               appended so the ctx matmul also emits the softmax denom)
  scores     : [k, q] psum; pb+mask bias accumulated into the same psum
               bank by an identity-matmul (out += I.T @ pbm), exp on ACT
               straight from psum. No max-subtraction: scores are O(50)
               so exp fits fp32/bf16 range, masked entries are -1e30 ->
               exp underflows to exact 0.
  ctx        : psum [65, q] f32; row 64 = sum_k exp  (denominator)
  normalize  : 1/denom as exp(-ln(x)) on ACT (batched per head-pair to
               avoid ACT function-table reloads), then gpsimd
               partition_broadcast + DVE mul -> ctxn bf16

Precision: Q/K path (projections + scores) in fp16 — scores feed exp(),
so absolute score error must stay ~1e-2; fp16 keeps it ~5e-3 while bf16
would give ~5e-2. V/ctx/output path errors only enter linearly, so bf16
is fine there (exp values overflow fp16 range, hence bf16 anyway).
V-projection packs two k-chunk accumulations per PSUM bank: the first
matmul's start=True clears has_written for the whole bank, the partner
k-chunk then starts with start=False and overwrites-where-unset.
The pb+mask add alternates between the PE (identity matmul) and the DVE
(tensor_add) by k-chunk parity to balance the two engines.
(Known-broken on HW, avoid: reciprocal_approx_fast and
partition_broadcast from a non-zero base partition — both return
garbage on silicon while passing CoreSim.)
"""

import sys

for _p in ("/opt/trn_rl_repo",):
    if _p not in sys.path:
        sys.path.insert(0, _p)

from contextlib import ExitStack

import numpy as np
import ml_dtypes

import concourse.bass as bass
import concourse.mybir as mybir
import concourse.tile as tile
from concourse import bacc
from concourse.bass_utils import run_bass_kernel_spmd

B, D, H, DH, LQ, LK = 2, 2048, 32, 64, 1024, 1024
NCORES = 8
HC = H // NCORES          # heads per core = 4
MR = HC * DH              # per-core model rows = 256
NEG = -1e30

DC = D // 128             # 16 d-chunks
NKC = LK // 128           # 8 k-chunks
NQB = LQ // 512           # 2 q blocks
NMC = MR // 128           # 2 dh-chunks
NOC = D // 128            # 16 output-row chunks

F32 = mybir.dt.float32
F32R = mybir.dt.float32r
F16 = mybir.dt.float16
BF16 = mybir.dt.bfloat16

USE_IDENT_ADD = True
CTX_LAG = 8
SPLIT_OUTPROJ = False

CFG = dict(
    dt_hid=F16,    # HBM hidden_q / hidden_kv (Q/K-path matmul operand)
    dt_w=F16,      # HBM Wq/Wk/Wv
    dt_wo=BF16,    # HBM Wo
    dt_pbm=BF16,   # HBM combined position_bias + mask bias
    dt_out=BF16,   # HBM partial output
    dt_qk=F16,     # SBUF h_q / h_k
    dt_v=BF16,     # SBUF h_vT / exp / ctxn
)

_NP = {F32: np.float32, F32R: np.float32, F16: np.float16,
       BF16: ml_dtypes.bfloat16}


def build_nc():
    dt_hid, dt_w, dt_wo = CFG["dt_hid"], CFG["dt_w"], CFG["dt_wo"]
    dt_pbm, dt_out = CFG["dt_pbm"], CFG["dt_out"]
    dt_qk, dt_v = CFG["dt_qk"], CFG["dt_v"]

    nc = bacc.Bacc("TRN2", target_bir_lowering=False, debug=False,
                   num_devices=NCORES)
    # hidden/pbm/weights are host-interleaved so each [128, 2*N] tile DMA
    # reads one contiguous 2*N-element run per partition (bigger packets)
    hq_e = nc.declare_dram_parameter("hq", [B, DC // 2, 128, 2 * LQ],
                                     dt_hid, False)
    hkv_e = nc.declare_dram_parameter("hkv", [B, DC // 2, 128, 2 * LK],
                                      dt_hid, False)
    pbm_e = nc.declare_dram_parameter("pbm", [B, HC, NKC // 2, 128, 2 * LQ],
                                      dt_pbm, False)
    wqt_e = nc.declare_dram_parameter("wqt", [DC // 4, 128, 4 * MR], dt_w,
                                      False)
    wkt_e = nc.declare_dram_parameter("wkt", [DC // 4, 128, 4 * MR], dt_w,
                                      False)
    wvt_e = nc.declare_dram_parameter("wvt", [DC // 4, 128, 4 * MR], dt_w,
                                      False)
    wot_e = nc.declare_dram_parameter("wot", [MR, D], dt_wo, False)
    id_e = nc.declare_dram_parameter("ident", [128, 128], dt_pbm, False)
    out_e = nc.declare_dram_parameter("out", [B, D, LQ], dt_out, True)
    hq_a, hkv_a, pbm_a, out_a = hq_e.ap(), hkv_e.ap(), pbm_e.ap(), out_e.ap()

    mm = nc.tensor.matmul
    Exp = mybir.ActivationFunctionType.Exp
    Ln = mybir.ActivationFunctionType.Ln

    with tile.TileContext(nc) as tc, ExitStack() as ctx:
        wp = ctx.enter_context(tc.tile_pool(name="w", bufs=1))
        hidp = ctx.enter_context(tc.tile_pool(name="hid", bufs=4))
        sbp = ctx.enter_context(tc.tile_pool(name="sb", bufs=1))
        pbmp = ctx.enter_context(tc.tile_pool(name="pbm", bufs=5))
        tmpp = ctx.enter_context(tc.tile_pool(name="tmp", bufs=4))
        expp = ctx.enter_context(tc.tile_pool(name="expp", bufs=8))
        psp = ctx.enter_context(tc.tile_pool(name="ps", bufs=4, space="PSUM"))

        # --- persistent weights, 4 d-chunks per DMA ----------------------
        # wX_sb[dc] are AP views into host-interleaved [128, 4*MR] tiles
        def load_w4(nm, ap_):
            views = []
            for g in range(DC // 4):
                t = wp.tile([128, 4 * MR], dt_w, tag=f"{nm}{g}", name=f"{nm}{g}")
                nc.sync.dma_start(t[:, :], ap_[g])
                for i in range(4):
                    views.append(t[:, i * MR:(i + 1) * MR])
            return views

        wk_sb, wv_sb = [], []
        pre_hkv, pre_pbm = {}, {}

        def wload(nm, ap_, lst, g):
            t = wp.tile([128, 4 * MR], dt_w, tag=f"{nm}{g}", name=f"{nm}{g}")
            nc.sync.dma_start(t[:, :], ap_[g])
            for i in range(4):
                lst.append(t[:, i * MR:(i + 1) * MR])

        def fetch_hkv(b, dc2, name):
            t = hidp.tile([128, 2 * LK], dt_hid, tag="hkv", name=name)
            nc.sync.dma_start(t[:, :], hkv_a[b, dc2 // 2])
            return t

        # startup: a 64 KB dc=0 slice of Wk and the first hidden pair
        # lead the DMA queue so the opening K matmuls start ASAP
        wk0s = wp.tile([128, MR], dt_w, tag="wk0s", name="wk0s")
        nc.sync.dma_start(wk0s[:, :], wkt_e.ap()[0, :, 0:MR])
        h0t = hidp.tile([128, 2 * LK], dt_hid, tag="hkv", name="hkvA0_0")
        nc.sync.dma_start(h0t[:, 0:LK], hkv_a[0, 0, :, 0:LK])
        nc.sync.dma_start(h0t[:, LK:2 * LK], hkv_a[0, 0, :, LK:2 * LK])
        pre_hkv[0] = h0t
        wload("wk", wkt_e.ap(), wk_sb, 0)
        wv0s = wp.tile([128, MR], dt_w, tag="wv0s", name="wv0s")
        nc.sync.dma_start(wv0s[:, :], wvt_e.ap()[0, :, 0:MR])
        wload("wv", wvt_e.ap(), wv_sb, 0)
        pre_hkv[2] = fetch_hkv(0, 2, "hkvA0_2")
        for g in range(1, DC // 4):
            wload("wk", wkt_e.ap(), wk_sb, g)
            wload("wv", wvt_e.ap(), wv_sb, g)
        wk_sb[0] = wk0s[:, :]
        wv_sb[0] = wv0s[:, :]
        ident = wp.tile([128, 128], dt_pbm, tag="ident", name="ident")
        nc.sync.dma_start(ident[:, :], id_e.ap()[:, :])
        ones1 = wp.tile([128, 1], F32, tag="ones1", name="ones1")
        nc.gpsimd.memset(ones1[:, :], 1.0)

        wq_sb, wo_sb = [], []

        def load_wq():
            wq_sb.extend(load_w4("wq", wqt_e.ap()))

        def load_wo():
            for c in range(NMC):
                t = wp.tile([128, D], dt_wo, tag=f"wo{c}", name=f"wo{c}")
                nc.sync.dma_start(t[:, :], wot_e.ap()[c * 128:(c + 1) * 128, :])
                wo_sb.append(t)

        ctxn_by_b = {}
        phases = {}

        for b in range(B):
            # --- pass A: K proj (4 banks) + V proj 2 k-chunks/bank (4) ---
            pk = [psp.tile([128, 512], F32, tag="big", name=f"pk{b}_{i}")
                  for i in range(4)]
            pv = [psp.tile([128, 512], F32, tag="pv", name=f"pv{b}_{i}")
                  for i in range(4)]
            hvT = [sbp.tile([128, HC * (DH + 1)], dt_v, tag=f"hv{kc}",
                            name=f"hv{b}_{kc}") for kc in range(NKC)]
            hk_sb = [sbp.tile([128, LK], dt_qk, tag=f"hk{mc}",
                              name=f"hk{b}_{mc}") for mc in range(NMC)]
            hq_sb = [sbp.tile([128, LQ], dt_qk, tag=f"hqs{mc}",
                              name=f"hqs{b}_{mc}") for mc in range(NMC)]

            for dc2 in range(0, DC, 2):
                if b == 0 and dc2 in pre_hkv:
                    hkv_t = pre_hkv.pop(dc2)
                else:
                    hkv_t = fetch_hkv(b, dc2, f"hkvA{b}_{dc2}")
                for i in range(2):
                    dc = dc2 + i
                    hv = hkv_t[:, i * LK:(i + 1) * LK]
                    for mc in range(NMC):
                        for kb in range(2):
                            mm(pk[mc * 2 + kb][:, :],
                               wk_sb[dc][:, mc * 128:(mc + 1) * 128],
                               hv[:, kb * 512:(kb + 1) * 512],
                               start=dc == 0, stop=dc == DC - 1)
                    for kc in range(NKC):
                        # two k-chunks share a psum bank; the first matmul
                        # clears the bank, the last one closes the group
                        mm(pv[kc // 2][:, (kc % 2) * MR:(kc % 2) * MR + MR],
                           hv[:, kc * 128:(kc + 1) * 128],
                           wv_sb[dc][:, :],
                           start=(dc == 0 and kc % 2 == 0),
                           stop=(dc == DC - 1 and kc % 2 == 1))
            for mc in range(NMC):
                for kb in range(2):
                    nc.vector.tensor_copy(hk_sb[mc][:, kb * 512:(kb + 1) * 512],
                                          pk[mc * 2 + kb][:, :])
            for kc in range(NKC):
                src = pv[kc // 2][:, (kc % 2) * MR:(kc % 2) * MR + MR]
                for h in range(HC):
                    nc.vector.tensor_copy(
                        hvT[kc][:, h * (DH + 1):h * (DH + 1) + DH],
                        src[:, h * DH:(h + 1) * DH])
                    nc.vector.tensor_copy(
                        hvT[kc][:, h * (DH + 1) + DH:(h + 1) * (DH + 1)],
                        ones1[:, :])

            # --- pass B: Q proj ------------------------------------------
            if b == 0:
                load_wq()
            # prefetch the first head-pair's pbm tiles so attention
            # doesn't stall on DMA right after the Q projection
            for h0 in range(2):
                t = pbmp.tile([128, 2 * LQ], dt_pbm, tag="pbm",
                              name=f"pbm{b}_{h0}_0p")
                nc.sync.dma_start(t[:, :], pbm_a[b, h0, 0])
                pre_pbm[(b, h0, 0)] = t
            pq = [psp.tile([128, 512], F32, tag="big", name=f"pq{b}_{i}")
                  for i in range(4)]
            for dc2 in range(0, DC, 2):
                hq_t = hidp.tile([128, 2 * LQ], dt_hid, tag="hq",
                                 name=f"hqB{b}_{dc2}")
                nc.sync.dma_start(hq_t[:, :], hq_a[b, dc2 // 2])
                for i in range(2):
                    dc = dc2 + i
                    hv = hq_t[:, i * LQ:(i + 1) * LQ]
                    for mc in range(NMC):
                        for qb in range(2):
                            mm(pq[mc * 2 + qb][:, :],
                               wq_sb[dc][:, mc * 128:(mc + 1) * 128],
                               hv[:, qb * 512:(qb + 1) * 512],
                               start=dc == 0, stop=dc == DC - 1)
            for mc in range(NMC):
                for qb in range(2):
                    nc.vector.tensor_copy(hq_sb[mc][:, qb * 512:(qb + 1) * 512],
                                          pq[mc * 2 + qb][:, :])

            # --- attention, head pairs ------------------------------------
            ctxn = [sbp.tile([128, LQ], dt_v, tag=f"ctxn{b}_{c}",
                             name=f"ctxn{b}_{c}") for c in range(NMC)]
            oacc = {}
            cxr = {}
            for hp in range(HC // 2):
                hc = hp
                heads = (2 * hp, 2 * hp + 1)
                pctx = {}
                for h in heads:
                    for qb in range(NQB):
                        pctx[(h, qb)] = psp.tile([DH + 1, 512], F32, tag="pv",
                                                 name=f"pctx{b}_{h}_{qb}")
                # ctx matmuls are emitted LAG units behind the score/exp
                # stream, so at a pair boundary the PE has fresh scores to
                # chew on instead of stalling on the previous pair's
                # normalize freeing a pctx slot
                pending = []

                def flush_ctx(limit):
                    while len(pending) > limit:
                        h2, qb2, kc_2, ex2 = pending.pop(0)
                        mm(pctx[(h2, qb2)][:, :],
                           hvT[kc_2][:, h2 * (DH + 1):(h2 + 1) * (DH + 1)],
                           ex2[:, :],
                           start=kc_2 == 0, stop=kc_2 == NKC - 1)

                for kc2 in range(0, NKC, 2):
                    pbm_t = {}
                    for h in heads:
                        key = (b, h, kc2)
                        if key in pre_pbm:
                            pbm_t[h] = pre_pbm.pop(key)
                        else:
                            pbm_t[h] = pbmp.tile([128, 2 * LQ], dt_pbm,
                                                 tag="pbm",
                                                 name=f"pbm{b}_{h}_{kc2}")
                            nc.sync.dma_start(pbm_t[h][:, :],
                                              pbm_a[b, h, kc2 // 2])
                    for i in range(2):
                        kc = kc2 + i
                        use_ident = USE_IDENT_ADD and kc % 2 == 0
                        for qb in range(NQB):
                            ps_t, ex_t = {}, {}
                            for h in heads:
                                po = (h % 2) * 64
                                ps_t[h] = psp.tile([128, 512], F32, tag="big",
                                                   name=f"ps{b}_{h}_{kc}_{qb}")
                                mm(ps_t[h][:, :],
                                   hk_sb[hc][po:po + 64,
                                             kc * 128:(kc + 1) * 128],
                                   hq_sb[hc][po:po + 64,
                                             qb * 512:(qb + 1) * 512],
                                   start=True, stop=not use_ident)
                            for h in heads:
                                if use_ident:
                                    mm(ps_t[h][:, :], ident[:, :],
                                       pbm_t[h][:, i * LQ + qb * 512:
                                                i * LQ + (qb + 1) * 512],
                                       start=False, stop=True)
                            for h in heads:
                                ex_t[h] = expp.tile([128, 512], dt_v, tag="exp",
                                                    bufs=8,
                                                    name=f"ex{b}_{h}_{kc}_{qb}")
                                if use_ident:
                                    nc.scalar.activation(ex_t[h][:, :],
                                                         ps_t[h][:, :], Exp)
                                else:
                                    tmq = expp.tile([128, 512], F32, tag="tmq",
                                                    bufs=6,
                                                    name=f"tq{b}_{h}_{kc}_{qb}")
                                    nc.vector.tensor_add(
                                        tmq[:, :], ps_t[h][:, :],
                                        pbm_t[h][:, i * LQ + qb * 512:
                                                 i * LQ + (qb + 1) * 512])
                                    nc.scalar.activation(ex_t[h][:, :],
                                                         tmq[:, :], Exp)
                            for h in heads:
                                pending.append((h, qb, kc, ex_t[h]))
                            flush_ctx(CTX_LAG)
                flush_ctx(0)
                # evacuate raw ctx+denominator to SBUF (~0.7us/copy),
                # freeing the pctx psum slots for the next pair immediately;
                # all normalization is deferred to the end of the batch
                # where it hides under the next batch's projections (or the
                # previous batch's deferred out-projection)
                for h in heads:
                    for qb in range(NQB):
                        t = tmpp.tile([DH + 1, 512], F32, tag="cxr", bufs=8,
                                      name=f"cxr{b}_{h}_{qb}")
                        nc.vector.tensor_copy(t[:, :], pctx[(h, qb)][:, :])
                        cxr[(h, qb)] = t

            # batch-end normalize: ctx[0:64] * (1 / ctx[64]); 1/x as
            # exp(-ln(x)) on ACT, Ln/Exp batched (2 table loads per batch)
            hqbs = [(h, qb) for qb in range(NQB) for h in range(HC)]
            rl_t, rc_t = {}, {}
            for j, (h, qb) in enumerate(hqbs):
                rl_t[j] = tmpp.tile([1, 512], F32, tag="rl", bufs=8,
                                    name=f"rl{b}_{h}_{qb}")
                nc.scalar.activation(rl_t[j][:, :],
                                     cxr[(h, qb)][DH:DH + 1, :], Ln)
            for j, (h, qb) in enumerate(hqbs):
                rc_t[j] = tmpp.tile([1, 512], F32, tag="rc", bufs=8,
                                    name=f"rc{b}_{h}_{qb}")
                nc.scalar.activation(rc_t[j][:, :], rl_t[j][:, :], Exp,
                                     scale=-1.0)
            for j, (h, qb) in enumerate(hqbs):
                po = (h % 2) * 64
                bc = tmpp.tile([64, 512], F32, tag="bc", bufs=6,
                               name=f"bc{b}_{h}_{qb}")
                nc.gpsimd.partition_broadcast(bc[:, :], rc_t[j][:, :])
                nc.vector.tensor_mul(
                    ctxn[(h // 2)][po:po + 64, qb * 512:(qb + 1) * 512],
                    cxr[(h, qb)][0:DH, :], bc[:, :])

            ctxn_by_b[b] = ctxn
            if b == 0:
                load_wo()

        def do_outproj(b):
            ctxn = ctxn_by_b[b]
            for oc in range(NOC):
                osb = tmpp.tile([128, LQ], dt_out, tag="osb",
                                name=f"osb{b}_{oc}")
                for qb in range(NQB):
                    po_t = psp.tile([128, 512], F32, tag="big",
                                    name=f"po{b}_{oc}_{qb}")
                    for c in range(NMC):
                        mm(po_t[:, :],
                           wo_sb[c][:, oc * 128:(oc + 1) * 128],
                           ctxn[c][:, qb * 512:(qb + 1) * 512],
                           start=c == 0, stop=c == NMC - 1)
                    nc.vector.tensor_copy(osb[:, qb * 512:(qb + 1) * 512],
                                          po_t[:, :])
                nc.sync.dma_start(out_a[b, oc * 128:(oc + 1) * 128, :],
                                  osb[:, :])

        for b in range(B):
            do_outproj(b)

    nc.compile()
    return nc


_NC_CACHE = None


def _get_nc():
    global _NC_CACHE
    if _NC_CACHE is None:
        _NC_CACHE = build_nc()
    return _NC_CACHE


def make_in_maps(hidden_q, hidden_kv, mask, position_bias, Wq, Wk, Wv, Wo):
    np_hid = _NP[CFG["dt_hid"]]
    np_w = _NP[CFG["dt_w"]]
    np_wo = _NP[CFG["dt_wo"]]
    np_pbm = _NP[CFG["dt_pbm"]]
    hidden_q = np.asarray(hidden_q, np.float32)
    hidden_kv = np.asarray(hidden_kv, np.float32)
    mask = np.asarray(mask)
    position_bias = np.asarray(position_bias, np.float32)
    Wq, Wk, Wv, Wo = (np.asarray(w, np.float32) for w in (Wq, Wk, Wv, Wo))

    maskb = np.where(mask != 0, np.float32(0), np.float32(NEG))  # [B, LK, LQ]

    def ilv_hid(x):
        # [B, D, L] -> [B, D/256, 128, 2*L]: partition p of chunk-pair g
        # holds rows g*256+p and g*256+128+p contiguously
        b_, d_, l_ = x.shape
        return np.ascontiguousarray(
            x.reshape(b_, d_ // 256, 2, 128, l_).transpose(0, 1, 3, 2, 4)
        ).reshape(b_, d_ // 256, 128, 2 * l_)

    def ilv_w(wt):
        # [D, MR] -> [D/512, 128, 4*MR]
        d_, m_ = wt.shape
        return np.ascontiguousarray(
            wt.reshape(d_ // 512, 4, 128, m_).transpose(0, 2, 1, 3)
        ).reshape(d_ // 512, 128, 4 * m_)

    hq = ilv_hid(hidden_q.astype(np_hid))
    hkv = ilv_hid(hidden_kv.astype(np_hid))
    ident = np.eye(128, dtype=np_pbm)
    in_maps = []
    for c in range(NCORES):
        hs = slice(c * HC, (c + 1) * HC)
        rs = slice(c * MR, (c + 1) * MR)
        pbm = (position_bias[hs][None] + maskb[:, None]).astype(np_pbm)
        pbm = np.ascontiguousarray(
            pbm.reshape(B, HC, NKC // 2, 2, 128, LQ).transpose(0, 1, 2, 4, 3, 5)
        ).reshape(B, HC, NKC // 2, 128, 2 * LQ)
        in_maps.append({
            "hq": hq,
            "hkv": hkv,
            "pbm": pbm,
            "wqt": ilv_w(np.ascontiguousarray(Wq[rs].T).astype(np_w)),
            "wkt": ilv_w(np.ascontiguousarray(Wk[rs].T).astype(np_w)),
            "wvt": ilv_w(np.ascontiguousarray(Wv[rs].T).astype(np_w)),
            "wot": np.ascontiguousarray(Wo[:, rs].T).astype(np_wo),
            "ident": ident,
        })
    return in_maps


def run(in_maps, trace=False):
    nc = _get_nc()
    return run_bass_kernel_spmd(nc, in_maps, core_ids=list(range(NCORES)),
                                trace=trace)


def kernel(hidden_q, hidden_kv, mask, position_bias, Wq, Wk, Wv, Wo):
    in_maps = make_in_maps(hidden_q, hidden_kv, mask, position_bias,
                           Wq, Wk, Wv, Wo)
    res = run(in_maps, trace=False)
    acc = np.zeros((B, D, LQ), np.float32)
    for r in res.results:
        acc += np.asarray(r["out"], dtype=np.float32)
    return acc


# revision 67
# speedup vs baseline: 1.0636x; 1.0017x over previous
"""Distributed sparse-attention kernel for 8 TRN2 NeuronCores.

Sharding: Megatron-style head parallelism. Core c owns heads [4c, 4c+4):
Wq/Wk/Wv column-parallel (rows of the [H*DH, D] weights), Wo row-parallel
(columns of [D, H*DH]). Each core computes a partial output
out_c = Wo_c @ ctx_c over its heads; the host sums the 8 partials.

Device layout choices (no on-chip transposes anywhere):
  h_q, h_k   : [dh, len] fp16  (proj psum M=dh-chunk, N=len)
  h_vT       : [k, dh+1] bf16  (proj psum M=k-chunk, N=dh; ones column
               appended so the ctx matmul also emits the softmax denom)
  scores     : [k, q] psum; pb+mask bias accumulated into the same psum
               bank by an identity-matmul (out += I.T @ pbm), exp on ACT
               straight from psum. No max-subtraction: scores are O(50)
               so exp fits fp32/bf16 range, masked entries are -1e30 ->
               exp underflows to exact 0.
  ctx        : psum [65, q] f32; row 64 = sum_k exp  (denominator)
  normalize  : 1/denom as exp(-ln(x)) on ACT (batched per head-pair to
               avoid ACT function-table reloads), then gpsimd
               partition_broadcast + DVE mul -> ctxn bf16

Precision: Q/K path (projections + scores) in fp16 — scores feed exp(),
so absolute score error must stay ~1e-2; fp16 keeps it ~5e-3 while bf16
would give ~5e-2. V/ctx/output path errors only enter linearly, so bf16
is fine there (exp values overflow fp16 range, hence bf16 anyway).
V-projection packs two k-chunk accumulations per PSUM bank: the first
matmul's start=True clears has_written for the whole bank, the partner
k-chunk then starts with start=False and overwrites-where-unset.
The pb+mask add alternates between the PE (identity matmul) and the DVE
(tensor_add) by k-chunk parity to balance the two engines.
(Known-broken on HW, avoid: reciprocal_approx_fast and
partition_broadcast from a non-zero base partition — both return
garbage on silicon while passing CoreSim.)
"""

import sys

for _p in ("/opt/trn_rl_repo",):
    if _p not in sys.path:
        sys.path.insert(0, _p)

from contextlib import ExitStack

import numpy as np
import ml_dtypes

import concourse.bass as bass
import concourse.mybir as mybir
import concourse.tile as tile
from concourse import bacc
from concourse.bass_utils import run_bass_kernel_spmd

B, D, H, DH, LQ, LK = 2, 2048, 32, 64, 1024, 1024
NCORES = 8
HC = H // NCORES          # heads per core = 4
MR = HC * DH              # per-core model rows = 256
NEG = -1e30

DC = D // 128             # 16 d-chunks
NKC = LK // 128           # 8 k-chunks
NQB = LQ // 512           # 2 q blocks
NMC = MR // 128           # 2 dh-chunks
NOC = D // 128            # 16 output-row chunks

F32 = mybir.dt.float32
F32R = mybir.dt.float32r
F16 = mybir.dt.float16
BF16 = mybir.dt.bfloat16

USE_IDENT_ADD = True
CTX_LAG = 8
SPLIT_OUTPROJ = False

CFG = dict(
    dt_hid=F16,    # HBM hidden_q / hidden_kv (Q/K-path matmul operand)
    dt_w=F16,      # HBM Wq/Wk/Wv
    dt_wo=BF16,    # HBM Wo
    dt_pbm=BF16,   # HBM combined position_bias + mask bias
    dt_out=BF16,   # HBM partial output
    dt_qk=F16,     # SBUF h_q / h_k
    dt_v=BF16,     # SBUF h_vT / exp / ctxn
)

_NP = {F32: np.float32, F32R: np.float32, F16: np.float16,
       BF16: ml_dtypes.bfloat16}


def build_nc():
    dt_hid, dt_w, dt_wo = CFG["dt_hid"], CFG["dt_w"], CFG["dt_wo"]
    dt_pbm, dt_out = CFG["dt_pbm"], CFG["dt_out"]
    dt_qk, dt_v = CFG["dt_qk"], CFG["dt_v"]

    nc = bacc.Bacc("TRN2", target_bir_lowering=False, debug=False,
                   num_devices=NCORES)
    # hidden/pbm/weights are host-interleaved so each [128, 2*N] tile DMA
    # reads one contiguous 2*N-element run per partition (bigger packets)
    hq_e = nc.declare_dram_parameter("hq", [B, DC // 2, 128, 2 * LQ],
                                     dt_hid, False)
    hkv_e = nc.declare_dram_parameter("hkv", [B, DC // 2, 128, 2 * LK],
                                      dt_hid, False)
    pbm_e = nc.declare_dram_parameter("pbm", [B, HC, NKC // 2, 128, 2 * LQ],
                                      dt_pbm, False)
    wqt_e = nc.declare_dram_parameter("wqt", [DC // 4, 128, 4 * MR], dt_w,
                                      False)
    wkt_e = nc.declare_dram_parameter("wkt", [DC // 4, 128, 4 * MR], dt_w,
                                      False)
    wvt_e = nc.declare_dram_parameter("wvt", [DC // 4, 128, 4 * MR], dt_w,
                                      False)
    wot_e = nc.declare_dram_parameter("wot", [MR, D], dt_wo, False)
    id_e = nc.declare_dram_parameter("ident", [128, 128], dt_pbm, False)
    out_e = nc.declare_dram_parameter("out", [B, D, LQ], dt_out, True)
    hq_a, hkv_a, pbm_a, out_a = hq_e.ap(), hkv_e.ap(), pbm_e.ap(), out_e.ap()

    mm = nc.tensor.matmul
    Exp = mybir.ActivationFunctionType.Exp
    Ln = mybir.ActivationFunctionType.Ln

    with tile.TileContext(nc) as tc, ExitStack() as ctx:
        wp = ctx.enter_context(tc.tile_pool(name="w", bufs=1))
        hidp = ctx.enter_context(tc.tile_pool(name="hid", bufs=4))
        sbp = ctx.enter_context(tc.tile_pool(name="sb", bufs=1))
        pbmp = ctx.enter_context(tc.tile_pool(name="pbm", bufs=5))
        tmpp = ctx.enter_context(tc.tile_pool(name="tmp", bufs=4))
        expp = ctx.enter_context(tc.tile_pool(name="expp", bufs=8))
        psp = ctx.enter_context(tc.tile_pool(name="ps", bufs=4, space="PSUM"))

        # --- persistent weights, 4 d-chunks per DMA ----------------------
        # wX_sb[dc] are AP views into host-interleaved [128, 4*MR] tiles
        def load_w4(nm, ap_):
            views = []
            for g in range(DC // 4):
                t = wp.tile([128, 4 * MR], dt_w, tag=f"{nm}{g}", name=f"{nm}{g}")
                nc.sync.dma_start(t[:, :], ap_[g])
                for i in range(4):
                    views.append(t[:, i * MR:(i + 1) * MR])
            return views

        wk_sb, wv_sb = [], []
        pre_hkv, pre_pbm = {}, {}

        def wload(nm, ap_, lst, g):
            t = wp.tile([128, 4 * MR], dt_w, tag=f"{nm}{g}", name=f"{nm}{g}")
            nc.sync.dma_start(t[:, :], ap_[g])
            for i in range(4):
                lst.append(t[:, i * MR:(i + 1) * MR])

        def fetch_hkv(b, dc2, name):
            t = hidp.tile([128, 2 * LK], dt_hid, tag="hkv", name=name)
            nc.sync.dma_start(t[:, :], hkv_a[b, dc2 // 2])
            return t

        # startup: a 64 KB dc=0 slice of Wk and the first hidden pair
        # lead the DMA queue so the opening K matmuls start ASAP
        wk0s = wp.tile([128, MR], dt_w, tag="wk0s", name="wk0s")
        nc.sync.dma_start(wk0s[:, :], wkt_e.ap()[0, :, 0:MR])
        h0t = hidp.tile([128, 2 * LK], dt_hid, tag="hkv", name="hkvA0_0")
        nc.sync.dma_start(h0t[:, 0:LK], hkv_a[0, 0, :, 0:LK])
        nc.sync.dma_start(h0t[:, LK:2 * LK], hkv_a[0, 0, :, LK:2 * LK])
        pre_hkv[0] = h0t
        wload("wk", wkt_e.ap(), wk_sb, 0)
        wv0s = wp.tile([128, MR], dt_w, tag="wv0s", name="wv0s")
        nc.sync.dma_start(wv0s[:, :], wvt_e.ap()[0, :, 0:MR])
        wload("wv", wvt_e.ap(), wv_sb, 0)
        pre_hkv[2] = fetch_hkv(0, 2, "hkvA0_2")
        for g in range(1, DC // 4):
            wload("wk", wkt_e.ap(), wk_sb, g)
            wload("wv", wvt_e.ap(), wv_sb, g)
        wk_sb[0] = wk0s[:, :]
        wv_sb[0] = wv0s[:, :]
        ident = wp.tile([128, 128], dt_pbm, tag="ident", name="ident")
        nc.sync.dma_start(ident[:, :], id_e.ap()[:, :])
        ones1 = wp.tile([128, 1], F32, tag="ones1", name="ones1")
        nc.gpsimd.memset(ones1[:, :], 1.0)

        wq_sb, wo_sb = [], []

        def load_wq():
            wq_sb.extend(load_w4("wq", wqt_e.ap()))

        def load_wo():
            for c in range(NMC):
                t = wp.tile([128, D], dt_wo, tag=f"wo{c}", name=f"wo{c}")
                nc.sync.dma_start(t[:, :], wot_e.ap()[c * 128:(c + 1) * 128, :])
                wo_sb.append(t)

        ctxn_by_b = {}
        phases = {}

        for b in range(B):
            # --- pass A: K proj (4 banks) + V proj 2 k-chunks/bank (4) ---
            pk = [psp.tile([128, 512], F32, tag="big", name=f"pk{b}_{i}")
                  for i in range(4)]
            pv = [psp.tile([128, 512], F32, tag="pv", name=f"pv{b}_{i}")
                  for i in range(4)]
            hvT = [sbp.tile([128, HC * (DH + 1)], dt_v, tag=f"hv{kc}",
                            name=f"hv{b}_{kc}") for kc in range(NKC)]
            hk_sb = [sbp.tile([128, LK], dt_qk, tag=f"hk{mc}",
                              name=f"hk{b}_{mc}") for mc in range(NMC)]
            hq_sb = [sbp.tile([128, LQ], dt_qk, tag=f"hqs{mc}",
                              name=f"hqs{b}_{mc}") for mc in range(NMC)]

            for dc2 in range(0, DC, 2):
                if b == 0 and dc2 in pre_hkv:
                    hkv_t = pre_hkv.pop(dc2)
                else:
                    hkv_t = fetch_hkv(b, dc2, f"hkvA{b}_{dc2}")
                for i in range(2):
                    dc = dc2 + i
                    hv = hkv_t[:, i * LK:(i + 1) * LK]
                    for mc in range(NMC):
                        for kb in range(2):
                            mm(pk[mc * 2 + kb][:, :],
                               wk_sb[dc][:, mc * 128:(mc + 1) * 128],
                               hv[:, kb * 512:(kb + 1) * 512],
                               start=dc == 0, stop=dc == DC - 1)
                    for kc in range(NKC):
                        # two k-chunks share a psum bank; the first matmul
                        # clears the bank, the last one closes the group
                        mm(pv[kc // 2][:, (kc % 2) * MR:(kc % 2) * MR + MR],
                           hv[:, kc * 128:(kc + 1) * 128],
                           wv_sb[dc][:, :],
                           start=(dc == 0 and kc % 2 == 0),
                           stop=(dc == DC - 1 and kc % 2 == 1))
            for mc in range(NMC):
                for kb in range(2):
                    nc.vector.tensor_copy(hk_sb[mc][:, kb * 512:(kb + 1) * 512],
                                          pk[mc * 2 + kb][:, :])
            for kc in range(NKC):
                src = pv[kc // 2][:, (kc % 2) * MR:(kc % 2) * MR + MR]
                for h in range(HC):
                    nc.vector.tensor_copy(
                        hvT[kc][:, h * (DH + 1):h * (DH + 1) + DH],
                        src[:, h * DH:(h + 1) * DH])
                    nc.vector.tensor_copy(
                        hvT[kc][:, h * (DH + 1) + DH:(h + 1) * (DH + 1)],
                        ones1[:, :])

            # --- pass B: Q proj ------------------------------------------
            if b == 0:
                load_wq()
            # prefetch the first head-pair's pbm tiles so attention
            # doesn't stall on DMA right after the Q projection
            for h0 in range(2):
                t = pbmp.tile([128, 2 * LQ], dt_pbm, tag="pbm",
                              name=f"pbm{b}_{h0}_0p")
                nc.sync.dma_start(t[:, :], pbm_a[b, h0, 0])
                pre_pbm[(b, h0, 0)] = t
            pq = [psp.tile([128, 512], F32, tag="big", name=f"pq{b}_{i}")
                  for i in range(4)]
            for dc2 in range(0, DC, 2):
                hq_t = hidp.tile([128, 2 * LQ], dt_hid, tag="hq",
                                 name=f"hqB{b}_{dc2}")
                nc.sync.dma_start(hq_t[:, :], hq_a[b, dc2 // 2])
                for i in range(2):
                    dc = dc2 + i
                    hv = hq_t[:, i * LQ:(i + 1) * LQ]
                    for mc in range(NMC):
                        for qb in range(2):
                            mm(pq[mc * 2 + qb][:, :],
                               wq_sb[dc][:, mc * 128:(mc + 1) * 128],
                               hv[:, qb * 512:(qb + 1) * 512],
                               start=dc == 0, stop=dc == DC - 1)
            for mc in range(NMC):
                for qb in range(2):
                    nc.vector.tensor_copy(hq_sb[mc][:, qb * 512:(qb + 1) * 512],
                                          pq[mc * 2 + qb][:, :])

            # --- attention, head pairs ------------------------------------
            ctxn = [sbp.tile([128, LQ], dt_v, tag=f"ctxn{b}_{c}",
                             name=f"ctxn{b}_{c}") for c in range(NMC)]
            oacc = {}
            cxr = {}
            for hp in range(HC // 2):
                hc = hp
                heads = (2 * hp, 2 * hp + 1)
                pctx = {}
                for h in heads:
                    for qb in range(NQB):
                        pctx[(h, qb)] = psp.tile([DH + 1, 512], F32, tag="pv",
                                                 name=f"pctx{b}_{h}_{qb}")
                # ctx matmuls are emitted LAG units behind the score/exp
                # stream, so at a pair boundary the PE has fresh scores to
                # chew on instead of stalling on the previous pair's
                # normalize freeing a pctx slot
                pending = []

                def flush_ctx(limit):
                    while len(pending) > limit:
                        h2, qb2, kc_2, ex2 = pending.pop(0)
                        mm(pctx[(h2, qb2)][:, :],
                           hvT[kc_2][:, h2 * (DH + 1):(h2 + 1) * (DH + 1)],
                           ex2[:, :],
                           start=kc_2 == 0, stop=kc_2 == NKC - 1)

                for kc2 in range(0, NKC, 2):
                    pbm_t = {}
                    for h in heads:
                        key = (b, h, kc2)
                        if key in pre_pbm:
                            pbm_t[h] = pre_pbm.pop(key)
                        else:
                            pbm_t[h] = pbmp.tile([128, 2 * LQ], dt_pbm,
                                                 tag="pbm",
                                                 name=f"pbm{b}_{h}_{kc2}")
                            nc.sync.dma_start(pbm_t[h][:, :],
                                              pbm_a[b, h, kc2 // 2])
                    for i in range(2):
                        kc = kc2 + i
                        use_ident = USE_IDENT_ADD and kc % 2 == 0
                        for qb in range(NQB):
                            ps_t, ex_t = {}, {}
                            for h in heads:
                                po = (h % 2) * 64
                                ps_t[h] = psp.tile([128, 512], F32, tag="big",
                                                   name=f"ps{b}_{h}_{kc}_{qb}")
                                mm(ps_t[h][:, :],
                                   hk_sb[hc][po:po + 64,
                                             kc * 128:(kc + 1) * 128],
                                   hq_sb[hc][po:po + 64,
                                             qb * 512:(qb + 1) * 512],
                                   start=True, stop=not use_ident)
                            for h in heads:
                                if use_ident:
                                    mm(ps_t[h][:, :], ident[:, :],
                                       pbm_t[h][:, i * LQ + qb * 512:
                                                i * LQ + (qb + 1) * 512],
                                       start=False, stop=True)
                            for h in heads:
                                ex_t[h] = expp.tile([128, 512], dt_v, tag="exp",
                                                    bufs=8,
                                                    name=f"ex{b}_{h}_{kc}_{qb}")
                                if use_ident:
                                    nc.scalar.activation(ex_t[h][:, :],
                                                         ps_t[h][:, :], Exp)
                                else:
                                    tmq = expp.tile([128, 512], F32, tag="tmq",
                                                    bufs=6,
                                                    name=f"tq{b}_{h}_{kc}_{qb}")
                                    nc.vector.tensor_add(
                                        tmq[:, :], ps_t[h][:, :],
                                        pbm_t[h][:, i * LQ + qb * 512:
                                                 i * LQ + (qb + 1) * 512])
                                    nc.scalar.activation(ex_t[h][:, :],
                                                         tmq[:, :], Exp)
                            for h in heads:
                                pending.append((h, qb, kc, ex_t[h]))
                            flush_ctx(CTX_LAG)
                flush_ctx(0)
                # evacuate raw ctx+denominator to SBUF (~0.7us/copy),
                # freeing the pctx psum slots for the next pair immediately;
                # all normalization is deferred to the end of the batch
                # where it hides under the next batch's projections (or the
                # previous batch's deferred out-projection)
                for h in heads:
                    for qb in range(NQB):
                        t = tmpp.tile([DH + 1, 512], F32, tag="cxr", bufs=8,
                                      name=f"cxr{b}_{h}_{qb}")
                        nc.vector.tensor_copy(t[:, :], pctx[(h, qb)][:, :])
                        cxr[(h, qb)] = t

            # batch-end normalize: ctx[0:64] * (1 / ctx[64]). qb=0 rows
            # via exp(-ln(x)) on ACT, qb=1 rows via exact reciprocal on the
            # (idle at batch end) DVE -- the two half-chains run on
            # different engines concurrently, halving the time until the
            # deferred out-projection unblocks.
            hqbs = [(h, qb) for qb in range(NQB) for h in range(HC)]
            rl_t, rc_t = {}, {}
            for j, (h, qb) in enumerate(hqbs):
                if qb == 0:
                    rl_t[j] = tmpp.tile([1, 512], F32, tag="rl", bufs=8,
                                        name=f"rl{b}_{h}_{qb}")
                    nc.scalar.activation(rl_t[j][:, :],
                                         cxr[(h, qb)][DH:DH + 1, :], Ln)
            for j, (h, qb) in enumerate(hqbs):
                rc_t[j] = tmpp.tile([1, 512], F32, tag="rc", bufs=8,
                                    name=f"rc{b}_{h}_{qb}")
                if qb == 0:
                    nc.scalar.activation(rc_t[j][:, :], rl_t[j][:, :], Exp,
                                         scale=-1.0)
                else:
                    nc.vector.reciprocal(rc_t[j][:, :],
                                         cxr[(h, qb)][DH:DH + 1, :])
            for j, (h, qb) in enumerate(hqbs):
                po = (h % 2) * 64
                bc = tmpp.tile([64, 512], F32, tag="bc", bufs=6,
                               name=f"bc{b}_{h}_{qb}")
                nc.gpsimd.partition_broadcast(bc[:, :], rc_t[j][:, :])
                nc.vector.tensor_mul(
                    ctxn[(h // 2)][po:po + 64, qb * 512:(qb + 1) * 512],
                    cxr[(h, qb)][0:DH, :], bc[:, :])

            ctxn_by_b[b] = ctxn
            if b == 0:
                load_wo()

        def do_outproj(b):
            ctxn = ctxn_by_b[b]
            for oc in range(NOC):
                osb = tmpp.tile([128, LQ], dt_out, tag="osb",
                                name=f"osb{b}_{oc}")
                for qb in range(NQB):
                    po_t = psp.tile([128, 512], F32, tag="big",
                                    name=f"po{b}_{oc}_{qb}")
                    for c in range(NMC):
                        mm(po_t[:, :],
                           wo_sb[c][:, oc * 128:(oc + 1) * 128],
                           ctxn[c][:, qb * 512:(qb + 1) * 512],
                           start=c == 0, stop=c == NMC - 1)
                    nc.vector.tensor_copy(osb[:, qb * 512:(qb + 1) * 512],
                                          po_t[:, :])
                nc.sync.dma_start(out_a[b, oc * 128:(oc + 1) * 128, :],
                                  osb[:, :])

        for b in range(B):
            do_outproj(b)

    nc.compile()
    return nc


_NC_CACHE = None


def _get_nc():
    global _NC_CACHE
    if _NC_CACHE is None:
        _NC_CACHE = build_nc()
    return _NC_CACHE


def make_in_maps(hidden_q, hidden_kv, mask, position_bias, Wq, Wk, Wv, Wo):
    np_hid = _NP[CFG["dt_hid"]]
    np_w = _NP[CFG["dt_w"]]
    np_wo = _NP[CFG["dt_wo"]]
    np_pbm = _NP[CFG["dt_pbm"]]
    hidden_q = np.asarray(hidden_q, np.float32)
    hidden_kv = np.asarray(hidden_kv, np.float32)
    mask = np.asarray(mask)
    position_bias = np.asarray(position_bias, np.float32)
    Wq, Wk, Wv, Wo = (np.asarray(w, np.float32) for w in (Wq, Wk, Wv, Wo))

    maskb = np.where(mask != 0, np.float32(0), np.float32(NEG))  # [B, LK, LQ]

    def ilv_hid(x):
        # [B, D, L] -> [B, D/256, 128, 2*L]: partition p of chunk-pair g
        # holds rows g*256+p and g*256+128+p contiguously
        b_, d_, l_ = x.shape
        return np.ascontiguousarray(
            x.reshape(b_, d_ // 256, 2, 128, l_).transpose(0, 1, 3, 2, 4)
        ).reshape(b_, d_ // 256, 128, 2 * l_)

    def ilv_w(wt):
        # [D, MR] -> [D/512, 128, 4*MR]
        d_, m_ = wt.shape
        return np.ascontiguousarray(
            wt.reshape(d_ // 512, 4, 128, m_).transpose(0, 2, 1, 3)
        ).reshape(d_ // 512, 128, 4 * m_)

    hq = ilv_hid(hidden_q.astype(np_hid))
    hkv = ilv_hid(hidden_kv.astype(np_hid))
    ident = np.eye(128, dtype=np_pbm)
    in_maps = []
    for c in range(NCORES):
        hs = slice(c * HC, (c + 1) * HC)
        rs = slice(c * MR, (c + 1) * MR)
        pbm = (position_bias[hs][None] + maskb[:, None]).astype(np_pbm)
        pbm = np.ascontiguousarray(
            pbm.reshape(B, HC, NKC // 2, 2, 128, LQ).transpose(0, 1, 2, 4, 3, 5)
        ).reshape(B, HC, NKC // 2, 128, 2 * LQ)
        in_maps.append({
            "hq": hq,
            "hkv": hkv,
            "pbm": pbm,
            "wqt": ilv_w(np.ascontiguousarray(Wq[rs].T).astype(np_w)),
            "wkt": ilv_w(np.ascontiguousarray(Wk[rs].T).astype(np_w)),
            "wvt": ilv_w(np.ascontiguousarray(Wv[rs].T).astype(np_w)),
            "wot": np.ascontiguousarray(Wo[:, rs].T).astype(np_wo),
            "ident": ident,
        })
    return in_maps


def run(in_maps, trace=False):
    nc = _get_nc()
    return run_bass_kernel_spmd(nc, in_maps, core_ids=list(range(NCORES)),
                                trace=trace)


def kernel(hidden_q, hidden_kv, mask, position_bias, Wq, Wk, Wv, Wo):
    in_maps = make_in_maps(hidden_q, hidden_kv, mask, position_bias,
                           Wq, Wk, Wv, Wo)
    res = run(in_maps, trace=False)
    acc = np.zeros((B, D, LQ), np.float32)
    for r in res.results:
        acc += np.asarray(r["out"], dtype=np.float32)
    return acc
